# revision 16
# baseline (speedup 1.0000x reference)
"""Trainium2 Bass kernel for nn_Average_Model_fwRF.

The whole model is a single linear functional of the inputs:

    out[b] = sum_l <fmap_l[b], mass_l (x) W_l> + s * sum(fc gathers * W) + bias
           = <X[b, :], V> + bias

so we fold the Gaussian masses and the [1,4200] linear weight into one
vector V (host side, tiny), pack each core's 64-batch slice of all the
fmaps/fcs into a d-major layout, and the device kernel is a streaming
dot product: for each [128 x 512] tile, matmul(lhsT=V-tile[128,8],
rhs=X-tile[128,512]) accumulating into one PSUM bank.  The 512-wide
free dim packs 8 d-groups x 64 batch; only the "diagonal" (group g of
the output row g) is extracted at the end.

Pure data parallel over batch: 8 cores x 64 batch, no collectives.
"""

import sys

if "/opt/trn_rl_repo" not in sys.path:
    sys.path.insert(0, "/opt/trn_rl_repo")

import numpy as np

B = 512
N_CORES = 8
BPC = B // N_CORES  # 64 batch per core
CONV = [(64, 27), (192, 27), (384, 13), (256, 13), (256, 13)]
FC_MAX = 1024
FC2 = 1000

D_CONV = sum(c * h * h for c, h in CONV)  # 338048
D_RAW = D_CONV + FC_MAX + FC_MAX + FC2  # 341096

G = 8  # d-groups per matmul; rhs free dim = G*BPC = 512
FREE = G * BPC  # 512
TW = G + FREE  # 520 columns per tile in the interleaved stream (8 V + 512 X)
D_PER_MM = G * 128  # 1024 d-values per matmul
NMM = -(-D_RAW // D_PER_MM)  # 334
TPC = 16  # matmuls per DMA chunk (chunk = ~2 MiB fp16)
NMM = -(-NMM // TPC) * TPC  # 336, pad to whole chunks
NCHUNK = NMM // TPC  # 21
DP = NMM * D_PER_MM  # 344064 padded feature dim
XBUFS = 5  # SBUF chunk buffers (deep prefetch keeps DMA from stalling)
WARM_MM = 12  # PE warm-up matmuls on scratch data at kernel start

PROFILE = False  # set by test.py (needs the ntff shim installed)
DTYPE = "float16"  # "float16" or "float32r" (device staging/matmul dtype)
VSCALE = np.float32(512.0)  # fp16: V pre-scaled by 2^9 to dodge subnormals
_CACHE = {}


def _np_dtype():
    return np.float16 if DTYPE == "float16" else np.float32


def _build():
    import concourse.tile as tile
    from concourse import bacc, mybir

    dt = getattr(mybir.dt, DTYPE)
    nc = bacc.Bacc("TRN2", debug=False, num_devices=N_CORES, enable_asserts=False)
    xv_d = nc.dram_tensor("xv", [128, NMM * TW], dt, kind="ExternalInput")
    out_d = nc.dram_tensor("o8", [G, FREE], mybir.dt.float32,
                           kind="ExternalOutput")

    with tile.TileContext(nc) as tc:
        with (
            tc.tile_pool(name="wp", bufs=1) as wp,
            tc.tile_pool(name="xp", bufs=XBUFS) as xp,
            tc.tile_pool(name="pp", bufs=1, space="PSUM") as pp,
            tc.tile_pool(name="wq", bufs=1, space="PSUM") as wq,
            tc.tile_pool(name="op", bufs=1) as op,
        ):
            # PE warm-up: ~5us of matmuls on scratch data so HAM reaches
            # K=8/8 while the first chunks are still in flight.
            wt = wp.tile([128, TW], dt)
            nc.gpsimd.memset(wt[:], 0.0)
            wps = wq.tile([G, FREE], mybir.dt.float32)
            for _ in range(WARM_MM):
                nc.tensor.matmul(wps[:], wt[:, :G], wt[:, G:], start=True,
                                 stop=True)

            ps = pp.tile([G, FREE], mybir.dt.float32)
            for c in range(NCHUNK):
                xt = xp.tile([128, TPC * TW], dt)
                # alternate the two HWDGE rings so per-chunk trigger latency
                # hides under the other ring's in-flight transfer
                eng = nc.sync if c % 2 == 0 else nc.scalar
                eng.dma_start(
                    xt[:], xv_d.ap()[:, c * TPC * TW:(c + 1) * TPC * TW]
                )
                for q in range(TPC):
                    t = c * TPC + q
                    nc.tensor.matmul(
                        ps[:],
                        xt[:, q * TW:q * TW + G],
                        xt[:, q * TW + G:(q + 1) * TW],
                        start=(t == 0),
                        stop=(t == NMM - 1),
                    )
            o8 = op.tile([G, FREE], mybir.dt.float32)
            nc.vector.tensor_copy(o8[:], ps[:])
            nc.sync.dma_start(out_d.ap()[:], o8[:])

    nc.compile()
    return nc


def _scale():
    return VSCALE if DTYPE == "float16" else np.float32(1.0)


def _build_v(mass, mfc, W, idx0, idx1):
    """Fold masses, the fc scalar, and W into one length-DP vector."""
    W = np.asarray(W, dtype=np.float32).reshape(-1) * _scale()
    s = np.float32(np.asarray(mfc).reshape(-1)[0])
    v = np.zeros(DP, dtype=np.float32)
    off_w = 0
    off_d = 0
    for (c, h), m in zip(CONV, mass):
        m = np.asarray(m, dtype=np.float32)
        v[off_d:off_d + c * h * h] = (
            W[off_w:off_w + c, None, None] * m[None, :, :]
        ).reshape(-1)
        off_w += c
        off_d += c * h * h
    for n in (FC_MAX, FC_MAX, FC2):
        v[off_d:off_d + n] = s * W[off_w:off_w + n]
        off_w += n
        off_d += n
    return v


def _pack_x(fmaps, fc0, fc1, fc2, idx0, idx1):
    """[B, D_RAW] activations -> per-core [128, NMM*FREE] d-major layout.

    Layout: col = t*FREE + g*64 + b, partition = p, holding
    X[core*64 + b, (t*G + g)*128 + p].
    """
    xall = np.zeros((B, DP), dtype=_np_dtype())
    off = 0
    for f in fmaps:
        f = np.asarray(f, dtype=np.float32)
        n = f.shape[1] * f.shape[2] * f.shape[3]
        xall[:, off:off + n] = f.reshape(B, n)
        off += n
    xall[:, off:off + FC_MAX] = np.asarray(fc0, dtype=np.float32).reshape(B, -1)[:, idx0]
    off += FC_MAX
    xall[:, off:off + FC_MAX] = np.asarray(fc1, dtype=np.float32).reshape(B, -1)[:, idx1]
    off += FC_MAX
    xall[:, off:off + FC2] = np.asarray(fc2, dtype=np.float32).reshape(B, -1)
    off += FC2
    assert off == D_RAW

    # [core, b, t, g, p] -> [core, p, t, g, b]
    return xall.reshape(N_CORES, BPC, NMM, G, 128).transpose(0, 4, 2, 3, 1)


def kernel(fmap0, fmap1, fmap2, fmap3, fmap4, fc0, fc1, fc2,
           mass0, mass1, mass2, mass3, mass4, mfc, W, b, idx0, idx1):
    from concourse.bass_utils import run_bass_kernel_spmd

    if "nc" not in _CACHE:
        _CACHE["nc"] = _build()
    nc = _CACHE["nc"]

    idx0 = np.asarray(idx0).astype(np.int64)
    idx1 = np.asarray(idx1).astype(np.int64)

    v = _build_v([mass0, mass1, mass2, mass3, mass4], mfc, W, idx0, idx1)
    vh = v.reshape(NMM, G, 128).transpose(2, 0, 1).astype(_np_dtype())  # [p,t,g]

    xh = _pack_x([fmap0, fmap1, fmap2, fmap3, fmap4], fc0, fc1, fc2, idx0, idx1)

    # interleaved stream: per tile t, 8 V columns then 512 X columns
    xv = np.empty((N_CORES, 128, NMM, TW), dtype=_np_dtype())
    xv[:, :, :, :G] = vh[None]
    for g in range(G):
        xv[:, :, :, G + g * BPC:G + (g + 1) * BPC] = xh[:, :, :, g, :]
    xv = xv.reshape(N_CORES, 128, NMM * TW)

    in_maps = [{"xv": xv[i]} for i in range(N_CORES)]

    res = run_bass_kernel_spmd(
        nc, in_maps, core_ids=list(range(N_CORES)), trace=PROFILE
    )
    if PROFILE and res.exec_time_ns is not None:
        print(f"HW exec time: {res.exec_time_ns} ns")
        _CACHE["exec_time_ns"] = res.exec_time_ns
        _CACHE["trace"] = res.instructions_and_trace

    bias = np.float32(np.asarray(b).reshape(-1)[0])
    inv_scale = np.float32(1.0) / _scale()
    out = np.empty((B, 1), dtype=np.float32)
    for i in range(N_CORES):
        o8 = res.results[i]["o8"].reshape(G, G, BPC)  # [g, g', b]
        diag = o8[np.arange(G), np.arange(G)]  # [G, BPC]
        out[i * BPC:(i + 1) * BPC, 0] = (
            diag.sum(axis=0, dtype=np.float32) * inv_scale + bias
        )
    return out


# revision 17
# speedup vs baseline: 1.4276x; 1.4276x over previous
"""Trainium2 Bass kernel for nn_Average_Model_fwRF.

The whole model is a single linear functional of the inputs:

    out[b] = sum_l <fmap_l[b], mass_l (x) W_l> + s * sum(fc gathers * W) + bias
           = <X[b, :], V> + bias

so we fold the Gaussian masses and the [1,4200] linear weight into one
vector V (host side, tiny), pack each core's 64-batch slice of the
activations into a d-major layout, and the device kernel is a streaming
dot product: for each [128 x 512] tile, matmul(lhsT=V-tile[128,8],
rhs=X-tile[128,512]) accumulating into one PSUM bank.  The 512-wide
free dim packs 8 d-groups x 64 batch; only the "diagonal" (group g of
output row g) is real — extracted on the host.

Mixed precision, driven by the error budget: the conv terms contribute
only ~2% of the output's magnitude (their folded weights mass*W are
tiny) but 99% of the bytes -> stream A carries them in fp8e4m3.  The
fc terms dominate the output but are only ~390 KB/core -> stream B in
fp16.  Each stream has its own PSUM accumulator and power-of-2 V
prescale (undone on the host), so fp8/fp16 subnormals are dodged.

Pure data parallel over batch: 8 cores x 64 batch, no collectives.
"""

import sys

if "/opt/trn_rl_repo" not in sys.path:
    sys.path.insert(0, "/opt/trn_rl_repo")

import numpy as np

B = 512
N_CORES = 8
BPC = B // N_CORES  # 64 batch per core
CONV = [(64, 27), (192, 27), (384, 13), (256, 13), (256, 13)]
FC_MAX = 1024
FC2 = 1000

D_CONV = sum(c * h * h for c, h in CONV)  # 338048
D_FC = FC_MAX + FC_MAX + FC2  # 3048

G = 8  # d-groups per matmul; rhs free dim = G*BPC = 512
FREE = G * BPC  # 512
TW = G + FREE  # 520 columns per tile in the interleaved stream (8 V + 512 X)
D_PER_MM = G * 128  # 1024 d-values per matmul

# stream A: conv activations, fp8e4m3
NMM_A = 336  # ceil(338048/1024)=331, padded to whole chunks
TPC_A = 28  # matmuls per DMA chunk
NCHUNK_A = NMM_A // TPC_A  # 12
DPA = NMM_A * D_PER_MM
VSCALE_A = np.float32(2.0 ** 15)  # mass*W products are ~1e-5; lift out of fp8 subnormals

# stream B: fc activations, fp16
NMM_B = 3  # ceil(3048/1024)
DPB = NMM_B * D_PER_MM
VSCALE_B = np.float32(2.0 ** 9)

XBUFS = 5  # SBUF chunk buffers for stream A
WARM_MM = 12  # PE warm-up matmuls on scratch data at kernel start

PROFILE = False  # set by test.py (needs the ntff shim installed)
_CACHE = {}


def _f8():
    from concourse import mybir

    return mybir.dt.np(mybir.dt.float8e4)


def _build():
    import concourse.tile as tile
    from concourse import bacc, mybir

    nc = bacc.Bacc("TRN2", debug=False, num_devices=N_CORES, enable_asserts=False)
    xva_d = nc.dram_tensor("xva", [128, NMM_A * TW], mybir.dt.float8e4,
                           kind="ExternalInput")
    xvb_d = nc.dram_tensor("xvb", [128, NMM_B * TW], mybir.dt.float16,
                           kind="ExternalInput")
    outa_d = nc.dram_tensor("oa", [G, FREE], mybir.dt.float32,
                            kind="ExternalOutput")
    outb_d = nc.dram_tensor("ob", [G, FREE], mybir.dt.float32,
                            kind="ExternalOutput")

    with tile.TileContext(nc) as tc:
        with (
            tc.tile_pool(name="wp", bufs=1) as wp,
            tc.tile_pool(name="bp", bufs=1) as bp,
            tc.tile_pool(name="xp", bufs=XBUFS) as xp,
            tc.tile_pool(name="pa", bufs=1, space="PSUM") as pa,
            tc.tile_pool(name="pb", bufs=1, space="PSUM") as pb,
            tc.tile_pool(name="wq", bufs=1, space="PSUM") as wq,
            tc.tile_pool(name="op", bufs=1) as op,
        ):
            # PE warm-up: ~5us of matmuls on scratch data so HAM reaches
            # K=8/8 while the first chunks are still in flight.
            wt = wp.tile([128, TW], mybir.dt.float8e4)
            nc.gpsimd.memset(wt[:], 0.0)
            wps = wq.tile([G, FREE], mybir.dt.float32)
            for _ in range(WARM_MM):
                nc.tensor.matmul(wps[:], wt[:, :G], wt[:, G:], start=True,
                                 stop=True)

            # stream B (fc, fp16): one small chunk, own accumulator
            xb = bp.tile([128, NMM_B * TW], mybir.dt.float16)
            nc.sync.dma_start(xb[:], xvb_d.ap()[:])
            psb = pb.tile([G, FREE], mybir.dt.float32)
            for t in range(NMM_B):
                nc.tensor.matmul(
                    psb[:],
                    xb[:, t * TW:t * TW + G],
                    xb[:, t * TW + G:(t + 1) * TW],
                    start=(t == 0),
                    stop=(t == NMM_B - 1),
                )

            # stream A (conv, fp8)
            psa = pa.tile([G, FREE], mybir.dt.float32)
            for c in range(NCHUNK_A):
                xt = xp.tile([128, TPC_A * TW], mybir.dt.float8e4)
                eng = nc.sync if c % 2 == 0 else nc.scalar
                eng.dma_start(
                    xt[:], xva_d.ap()[:, c * TPC_A * TW:(c + 1) * TPC_A * TW]
                )
                for q in range(TPC_A):
                    t = c * TPC_A + q
                    nc.tensor.matmul(
                        psa[:],
                        xt[:, q * TW:q * TW + G],
                        xt[:, q * TW + G:(q + 1) * TW],
                        start=(t == 0),
                        stop=(t == NMM_A - 1),
                    )

            o8a = op.tile([G, FREE], mybir.dt.float32)
            nc.vector.tensor_copy(o8a[:], psa[:])
            nc.sync.dma_start(outa_d.ap()[:], o8a[:])
            o8b = op.tile([G, FREE], mybir.dt.float32)
            nc.vector.tensor_copy(o8b[:], psb[:])
            nc.scalar.dma_start(outb_d.ap()[:], o8b[:])

    nc.compile()
    return nc


def _interleave(xall, vflat, n_mm, np_dt):
    """[B, DP] batch-major activations + [DP] V -> per-core interleaved
    [core, 128, n_mm*TW] stream: per tile t, 8 V columns then 512 X
    columns, with col = g*64+b and partition p holding d=(t*G+g)*128+p."""
    vh = vflat.reshape(n_mm, G, 128).transpose(2, 0, 1).astype(np_dt)  # [p,t,g]
    xh = xall.reshape(N_CORES, BPC, n_mm, G, 128).transpose(0, 4, 2, 3, 1)
    xv = np.empty((N_CORES, 128, n_mm, TW), dtype=np_dt)
    xv[:, :, :, :G] = vh[None]
    for g in range(G):
        xv[:, :, :, G + g * BPC:G + (g + 1) * BPC] = xh[:, :, :, g, :]
    return xv.reshape(N_CORES, 128, n_mm * TW)


def kernel(fmap0, fmap1, fmap2, fmap3, fmap4, fc0, fc1, fc2,
           mass0, mass1, mass2, mass3, mass4, mfc, W, b, idx0, idx1):
    from concourse.bass_utils import run_bass_kernel_spmd

    if "nc" not in _CACHE:
        _CACHE["nc"] = _build()
    nc = _CACHE["nc"]

    idx0 = np.asarray(idx0).astype(np.int64)
    idx1 = np.asarray(idx1).astype(np.int64)
    W_ = np.asarray(W, dtype=np.float32).reshape(-1)
    s = np.float32(np.asarray(mfc).reshape(-1)[0])

    # ---- stream A: conv activations + folded V = mass (x) W ----
    fmaps = [fmap0, fmap1, fmap2, fmap3, fmap4]
    masses = [mass0, mass1, mass2, mass3, mass4]
    f8 = _f8()
    xa = np.zeros((B, DPA), dtype=f8)
    va = np.zeros(DPA, dtype=np.float32)
    off_w = 0
    off_d = 0
    for (c, h), f, m in zip(CONV, fmaps, masses):
        n = c * h * h
        xa[:, off_d:off_d + n] = np.asarray(f, dtype=np.float32).reshape(B, n)
        m = np.asarray(m, dtype=np.float32)
        va[off_d:off_d + n] = (
            (VSCALE_A * W_[off_w:off_w + c])[:, None, None] * m[None, :, :]
        ).reshape(-1)
        off_w += c
        off_d += n
    xva = _interleave(xa, va, NMM_A, f8)

    # ---- stream B: gathered fc activations + V = s * W ----
    xb = np.zeros((B, DPB), dtype=np.float16)
    vb = np.zeros(DPB, dtype=np.float32)
    fcs = [(np.asarray(fc0, dtype=np.float32).reshape(B, -1)[:, idx0], FC_MAX),
           (np.asarray(fc1, dtype=np.float32).reshape(B, -1)[:, idx1], FC_MAX),
           (np.asarray(fc2, dtype=np.float32).reshape(B, -1), FC2)]
    off_d = 0
    for data, n in fcs:
        xb[:, off_d:off_d + n] = data
        vb[off_d:off_d + n] = VSCALE_B * s * W_[off_w:off_w + n]
        off_w += n
        off_d += n
    xvb = _interleave(xb, vb, NMM_B, np.float16)

    in_maps = [{"xva": xva[i], "xvb": xvb[i]} for i in range(N_CORES)]

    res = run_bass_kernel_spmd(
        nc, in_maps, core_ids=list(range(N_CORES)), trace=PROFILE
    )
    if PROFILE and res.exec_time_ns is not None:
        print(f"HW exec time: {res.exec_time_ns} ns")
        _CACHE["exec_time_ns"] = res.exec_time_ns
        _CACHE["trace"] = res.instructions_and_trace

    bias = np.float32(np.asarray(b).reshape(-1)[0])
    ia, ib = np.float32(1.0) / VSCALE_A, np.float32(1.0) / VSCALE_B
    rng = np.arange(G)
    out = np.empty((B, 1), dtype=np.float32)
    for i in range(N_CORES):
        da = res.results[i]["oa"].reshape(G, G, BPC)[rng, rng]
        db = res.results[i]["ob"].reshape(G, G, BPC)[rng, rng]
        out[i * BPC:(i + 1) * BPC, 0] = (
            da.sum(axis=0, dtype=np.float32) * ia
            + db.sum(axis=0, dtype=np.float32) * ib
            + bias
        )
    return out


# revision 18
# speedup vs baseline: 1.5324x; 1.0734x over previous
"""Trainium2 Bass kernel for nn_Average_Model_fwRF.

The whole model is a single linear functional of the inputs:

    out[b] = sum_l <fmap_l[b], mass_l (x) W_l> + s * sum(fc gathers * W) + bias
           = <X[b, :], V> + bias

so we fold the Gaussian masses and the [1,4200] linear weight into one
vector V (host side, tiny), pack each core's 64-batch slice of the
activations into a d-major layout, and the device kernel is a streaming
dot product on the TensorEngine, accumulating into one PSUM bank per
stream.  The 512-wide matmul free dim packs 8 d-groups x 64 batch; only
the "diagonal" (group g of output row g) is real — extracted on host.

Mixed precision, driven by the error budget: the conv terms contribute
only ~2% of the output's magnitude (their folded weights mass*W are
tiny) but 99% of the bytes -> stream A carries them in fp8e4m3 with
DoubleRow matmuls (2 fp8 elements per PE cell -> 2x contraction per
cycle).  The fc terms dominate the output but are only ~390 KB/core ->
stream B in fp16.  Each stream has its own PSUM accumulator and
power-of-2 V prescale (undone on the host) to dodge subnormals.

Pure data parallel over batch: 8 cores x 64 batch, no collectives.
"""

import sys

if "/opt/trn_rl_repo" not in sys.path:
    sys.path.insert(0, "/opt/trn_rl_repo")

import numpy as np

B = 512
N_CORES = 8
BPC = B // N_CORES  # 64 batch per core
CONV = [(64, 27), (192, 27), (384, 13), (256, 13), (256, 13)]
FC_MAX = 1024
FC2 = 1000

D_CONV = sum(c * h * h for c, h in CONV)  # 338048
D_FC = FC_MAX + FC_MAX + FC2  # 3048

G = 8  # d-groups per matmul; free dim = G*BPC = 512
FREE = G * BPC  # 512

# stream A: conv activations, fp8e4m3, DoubleRow matmuls.
# Per DoubleRow tile: 32 V cols ([i=2, m=16], m>=8 zero-padded so the
# i-stride is 16 B) + 1024 X cols ([i=2, n=512]); contracts 2048 d.
MD = 16  # stationary columns (8 real + 8 pad)
TWA = 2 * MD + 2 * FREE  # 1056
D_PER_A = 2 * G * 128  # 2048
NDR = 168  # ceil(338048/2048)=166, padded to whole chunks
DPA = NDR * D_PER_A  # 344064
CHUNKS_A = [3, 4, 7] + [14] * 11  # tiles per DMA chunk (staggered start)
assert sum(CHUNKS_A) == NDR
VSCALE_A = np.float32(2.0 ** 15)  # mass*W ~1e-5; lift out of fp8 subnormals

# stream B: fc activations, fp16
TWB = G + FREE  # 520
NMM_B = 3  # ceil(3048/1024)
DPB = NMM_B * G * 128
VSCALE_B = np.float32(2.0 ** 9)

XBUFS = 5  # SBUF chunk buffers for stream A
WARM_MM = 12  # PE warm-up matmuls on scratch data at kernel start

PROFILE = False  # set by test.py (needs the ntff shim installed)
_CACHE = {}


def _f8():
    from concourse import mybir

    return mybir.dt.np(mybir.dt.float8e4)


def _build():
    import concourse.tile as tile
    from concourse import bacc, mybir

    nc = bacc.Bacc("TRN2", debug=False, num_devices=N_CORES, enable_asserts=False)
    xva_d = nc.dram_tensor("xva", [128, NDR * TWA], mybir.dt.float8e4,
                           kind="ExternalInput")
    xvb_d = nc.dram_tensor("xvb", [128, NMM_B * TWB], mybir.dt.float16,
                           kind="ExternalInput")
    outa_d = nc.dram_tensor("oa", [MD, FREE], mybir.dt.float32,
                            kind="ExternalOutput")
    outb_d = nc.dram_tensor("ob", [G, FREE], mybir.dt.float32,
                            kind="ExternalOutput")

    with tile.TileContext(nc) as tc:
        with (
            tc.tile_pool(name="wp", bufs=1) as wp,
            tc.tile_pool(name="bp", bufs=1) as bp,
            tc.tile_pool(name="xp", bufs=XBUFS) as xp,
            tc.tile_pool(name="pa", bufs=1, space="PSUM") as pa,
            tc.tile_pool(name="pb", bufs=1, space="PSUM") as pb,
            tc.tile_pool(name="wq", bufs=1, space="PSUM") as wq,
            tc.tile_pool(name="op", bufs=1) as op,
        ):
            # PE warm-up: ~5us of matmuls on scratch data so HAM reaches
            # K=8/8 while the first chunks are still in flight.
            wt = wp.tile([128, TWB], mybir.dt.float8e4)
            nc.gpsimd.memset(wt[:], 0.0)
            wps = wq.tile([G, FREE], mybir.dt.float32)
            for _ in range(WARM_MM):
                nc.tensor.matmul(wps[:], wt[:, :G], wt[:, G:], start=True,
                                 stop=True)

            # stream B (fc, fp16): one small chunk, own accumulator
            xb = bp.tile([128, NMM_B * TWB], mybir.dt.float16)
            nc.sync.dma_start(xb[:], xvb_d.ap()[:])
            psb = pb.tile([G, FREE], mybir.dt.float32)
            for t in range(NMM_B):
                nc.tensor.matmul(
                    psb[:],
                    xb[:, t * TWB:t * TWB + G],
                    xb[:, t * TWB + G:(t + 1) * TWB],
                    start=(t == 0),
                    stop=(t == NMM_B - 1),
                )

            # stream A (conv, fp8, DoubleRow)
            psa = pa.tile([MD, FREE], mybir.dt.float32)
            tt = 0
            col = 0
            for c, ntiles in enumerate(CHUNKS_A):
                w = ntiles * TWA
                xt = xp.tile([128, max(CHUNKS_A) * TWA], mybir.dt.float8e4,
                             tag="xa")
                eng = nc.sync if c % 2 == 0 else nc.scalar
                eng.dma_start(xt[:, :w], xva_d.ap()[:, col:col + w])
                col += w
                for q in range(ntiles):
                    base = q * TWA
                    lhsT = xt[:, base:base + 2 * MD].rearrange(
                        "p (i m) -> p i m", i=2)
                    rhs = xt[:, base + 2 * MD:base + TWA].rearrange(
                        "p (i n) -> p i n", i=2)
                    nc.tensor.matmul(
                        psa[:], lhsT, rhs,
                        start=(tt == 0),
                        stop=(tt == NDR - 1),
                        perf_mode=mybir.MatmulPerfMode.DoubleRow,
                    )
                    tt += 1

            o8a = op.tile([MD, FREE], mybir.dt.float32)
            nc.vector.tensor_copy(o8a[:], psa[:])
            nc.sync.dma_start(outa_d.ap()[:], o8a[:])
            o8b = op.tile([G, FREE], mybir.dt.float32)
            nc.vector.tensor_copy(o8b[:], psb[:])
            nc.scalar.dma_start(outb_d.ap()[:], o8b[:])

    nc.compile()
    return nc


def kernel(fmap0, fmap1, fmap2, fmap3, fmap4, fc0, fc1, fc2,
           mass0, mass1, mass2, mass3, mass4, mfc, W, b, idx0, idx1):
    from concourse.bass_utils import run_bass_kernel_spmd

    if "nc" not in _CACHE:
        _CACHE["nc"] = _build()
    nc = _CACHE["nc"]

    idx0 = np.asarray(idx0).astype(np.int64)
    idx1 = np.asarray(idx1).astype(np.int64)
    W_ = np.asarray(W, dtype=np.float32).reshape(-1)
    s = np.float32(np.asarray(mfc).reshape(-1)[0])

    # ---- stream A: conv activations + folded V = mass (x) W ----
    # d = tt*2048 + g*256 + i*128 + p
    fmaps = [fmap0, fmap1, fmap2, fmap3, fmap4]
    masses = [mass0, mass1, mass2, mass3, mass4]
    f8 = _f8()
    xa = np.zeros((B, DPA), dtype=f8)
    va = np.zeros(DPA, dtype=np.float32)
    off_w = 0
    off_d = 0
    for (c, h), f, m in zip(CONV, fmaps, masses):
        n = c * h * h
        xa[:, off_d:off_d + n] = np.asarray(f, dtype=np.float32).reshape(B, n)
        m = np.asarray(m, dtype=np.float32)
        va[off_d:off_d + n] = (
            (VSCALE_A * W_[off_w:off_w + c])[:, None, None] * m[None, :, :]
        ).reshape(-1)
        off_w += c
        off_d += n

    xva = np.empty((N_CORES, 128, NDR, TWA), dtype=f8)
    vblk = np.zeros((128, NDR, 2, MD), dtype=f8)
    vblk[:, :, :, :G] = va.reshape(NDR, G, 2, 128).transpose(3, 0, 2, 1)
    xva[:, :, :, :2 * MD] = vblk.reshape(128, NDR, 2 * MD)[None]
    xsrc = xa.reshape(N_CORES, BPC, NDR, G, 2, 128).transpose(0, 5, 2, 4, 3, 1)
    for i in range(2):
        for g in range(G):
            c0 = 2 * MD + i * FREE + g * BPC
            xva[:, :, :, c0:c0 + BPC] = xsrc[:, :, :, i, g, :]
    xva = xva.reshape(N_CORES, 128, NDR * TWA)

    # ---- stream B: gathered fc activations + V = s * W ----
    xb = np.zeros((B, DPB), dtype=np.float16)
    vb = np.zeros(DPB, dtype=np.float32)
    fcs = [(np.asarray(fc0, dtype=np.float32).reshape(B, -1)[:, idx0], FC_MAX),
           (np.asarray(fc1, dtype=np.float32).reshape(B, -1)[:, idx1], FC_MAX),
           (np.asarray(fc2, dtype=np.float32).reshape(B, -1), FC2)]
    off_d = 0
    for data, n in fcs:
        xb[:, off_d:off_d + n] = data
        vb[off_d:off_d + n] = VSCALE_B * s * W_[off_w:off_w + n]
        off_w += n
        off_d += n
    vhb = vb.reshape(NMM_B, G, 128).transpose(2, 0, 1).astype(np.float16)
    xhb = xb.reshape(N_CORES, BPC, NMM_B, G, 128).transpose(0, 4, 2, 3, 1)
    xvb = np.empty((N_CORES, 128, NMM_B, TWB), dtype=np.float16)
    xvb[:, :, :, :G] = vhb[None]
    for g in range(G):
        xvb[:, :, :, G + g * BPC:G + (g + 1) * BPC] = xhb[:, :, :, g, :]
    xvb = xvb.reshape(N_CORES, 128, NMM_B * TWB)

    in_maps = [{"xva": xva[i], "xvb": xvb[i]} for i in range(N_CORES)]

    res = run_bass_kernel_spmd(
        nc, in_maps, core_ids=list(range(N_CORES)), trace=PROFILE
    )
    if PROFILE and res.exec_time_ns is not None:
        print(f"HW exec time: {res.exec_time_ns} ns")
        _CACHE["exec_time_ns"] = res.exec_time_ns
        _CACHE["trace"] = res.instructions_and_trace

    bias = np.float32(np.asarray(b).reshape(-1)[0])
    ia, ib = np.float32(1.0) / VSCALE_A, np.float32(1.0) / VSCALE_B
    rng = np.arange(G)
    out = np.empty((B, 1), dtype=np.float32)
    for i in range(N_CORES):
        da = res.results[i]["oa"].reshape(MD, G, BPC)[rng, rng]
        db = res.results[i]["ob"].reshape(G, G, BPC)[rng, rng]
        out[i * BPC:(i + 1) * BPC, 0] = (
            da.sum(axis=0, dtype=np.float32) * ia
            + db.sum(axis=0, dtype=np.float32) * ib
            + bias
        )
    return out


# revision 20
# speedup vs baseline: 1.7182x; 1.1212x over previous
"""Trainium2 Bass kernel for nn_Average_Model_fwRF.

The whole model is a single linear functional of the inputs:

    out[b] = sum_l <fmap_l[b], mass_l (x) W_l> + s * sum(fc gathers * W) + bias
           = <X[b, :], V> + bias

so we fold the Gaussian masses and the [1,4200] linear weight into one
vector V (host side, tiny), pack each core's 64-batch slice of the
activations into a d-major layout, and the device kernel is a streaming
dot product on the TensorEngine, accumulating into one PSUM bank per
stream.  The 512-wide matmul free dim packs 8 d-groups x 64 batch; only
the "diagonal" (group g of output row g) is real — extracted on host.

Mixed precision, driven by the error budget: the conv terms contribute
only ~2% of the output's magnitude (their folded weights mass*W are
tiny) but 99% of the bytes -> stream A carries them in fp8e4m3 with
DoubleRow matmuls (2 fp8 elements per PE cell -> 2x contraction per
cycle).  The fc terms dominate the output but are only ~390 KB/core ->
stream B in fp16.  Each stream has its own PSUM accumulator and
power-of-2 V prescale (undone on the host) to dodge subnormals.

Pure data parallel over batch: 8 cores x 64 batch, no collectives.
"""

import sys

if "/opt/trn_rl_repo" not in sys.path:
    sys.path.insert(0, "/opt/trn_rl_repo")

import numpy as np

B = 512
N_CORES = 8
BPC = B // N_CORES  # 64 batch per core
CONV = [(64, 27), (192, 27), (384, 13), (256, 13), (256, 13)]
FC_MAX = 1024
FC2 = 1000

D_CONV = sum(c * h * h for c, h in CONV)  # 338048
D_FC = FC_MAX + FC_MAX + FC2  # 3048

G = 8  # d-groups per matmul; free dim = G*BPC = 512
FREE = G * BPC  # 512

# stream A: conv activations, fp8e4m3, DoubleRow matmuls.
# Per DoubleRow tile: 32 V cols ([i=2, m=16], m>=8 zero-padded so the
# i-stride is 16 B) + 1024 X cols ([i=2, n=512]); contracts 2048 d.
MD = 16  # stationary columns (8 real + 8 pad)
TWA = 2 * MD + 2 * FREE  # 1056
D_PER_A = 2 * G * 128  # 2048
NDR = 166  # ceil(338048/2048)
DPA = NDR * D_PER_A  # 339968
# tiles per DMA chunk: small chunks first (low latency to first matmul) and
# last (so the final chunk's matmuls barely outlive the DMA stream)
CHUNKS_A = [3, 4, 7] + [14] * 9 + [9, 7, 5, 3, 2]
assert sum(CHUNKS_A) == NDR
VSCALE_A = np.float32(2.0 ** 15)  # mass*W ~1e-5; lift out of fp8 subnormals

# stream B: fc activations, fp16
TWB = G + FREE  # 520
NMM_B = 3  # ceil(3048/1024)
DPB = NMM_B * G * 128
VSCALE_B = np.float32(2.0 ** 9)

XBUFS = 8  # SBUF chunk buffers for stream A (deep prefetch rides out
           # HBM-stack contention bursts from the paired NeuronCore)
WARM_MM = 8  # PE warm-up matmuls on scratch data at kernel start

PROFILE = False  # set by test.py (needs the ntff shim installed)
_CACHE = {}


def _f8():
    from concourse import mybir

    return mybir.dt.np(mybir.dt.float8e4)


def _build():
    import concourse.tile as tile
    from concourse import bacc, mybir

    nc = bacc.Bacc("TRN2", debug=False, num_devices=N_CORES, enable_asserts=False)
    xva_d = nc.dram_tensor("xva", [128, NDR * TWA], mybir.dt.float8e4,
                           kind="ExternalInput")
    xvb_d = nc.dram_tensor("xvb", [128, NMM_B * TWB], mybir.dt.float16,
                           kind="ExternalInput")
    outa_d = nc.dram_tensor("oa", [MD, FREE], mybir.dt.float32,
                            kind="ExternalOutput")
    outb_d = nc.dram_tensor("ob", [G, FREE], mybir.dt.float32,
                            kind="ExternalOutput")

    with tile.TileContext(nc) as tc:
        with (
            tc.tile_pool(name="wp", bufs=1) as wp,
            tc.tile_pool(name="bp", bufs=1) as bp,
            tc.tile_pool(name="xp", bufs=XBUFS) as xp,
            tc.tile_pool(name="pa", bufs=1, space="PSUM") as pa,
            tc.tile_pool(name="pb", bufs=1, space="PSUM") as pb,
            tc.tile_pool(name="wq", bufs=1, space="PSUM") as wq,
            tc.tile_pool(name="op", bufs=1) as op,
        ):
            # PE warm-up: ~5us of matmuls on scratch data so HAM reaches
            # K=8/8 while the first chunks are still in flight.
            wt = wp.tile([128, TWB], mybir.dt.float8e4)
            nc.gpsimd.memset(wt[:], 0.0)
            wps = wq.tile([G, FREE], mybir.dt.float32)
            for _ in range(WARM_MM):
                nc.tensor.matmul(wps[:], wt[:, :G], wt[:, G:], start=True,
                                 stop=True)

            # stream B (fc, fp16): one small chunk, own accumulator
            xb = bp.tile([128, NMM_B * TWB], mybir.dt.float16)
            nc.sync.dma_start(xb[:], xvb_d.ap()[:])
            psb = pb.tile([G, FREE], mybir.dt.float32)
            for t in range(NMM_B):
                nc.tensor.matmul(
                    psb[:],
                    xb[:, t * TWB:t * TWB + G],
                    xb[:, t * TWB + G:(t + 1) * TWB],
                    start=(t == 0),
                    stop=(t == NMM_B - 1),
                )

            # stream A (conv, fp8, DoubleRow)
            psa = pa.tile([MD, FREE], mybir.dt.float32)
            tt = 0
            col = 0
            for c, ntiles in enumerate(CHUNKS_A):
                w = ntiles * TWA
                xt = xp.tile([128, max(CHUNKS_A) * TWA], mybir.dt.float8e4,
                             tag="xa")
                eng = nc.sync if c % 2 == 0 else nc.scalar
                eng.dma_start(xt[:, :w], xva_d.ap()[:, col:col + w])
                col += w
                for q in range(ntiles):
                    base = q * TWA
                    lhsT = xt[:, base:base + 2 * MD].rearrange(
                        "p (i m) -> p i m", i=2)
                    rhs = xt[:, base + 2 * MD:base + TWA].rearrange(
                        "p (i n) -> p i n", i=2)
                    nc.tensor.matmul(
                        psa[:], lhsT, rhs,
                        start=(tt == 0),
                        stop=(tt == NDR - 1),
                        perf_mode=mybir.MatmulPerfMode.DoubleRow,
                    )
                    tt += 1

            o8a = op.tile([MD, FREE], mybir.dt.float32)
            nc.vector.tensor_copy(o8a[:], psa[:])
            nc.sync.dma_start(outa_d.ap()[:], o8a[:])
            o8b = op.tile([G, FREE], mybir.dt.float32)
            nc.vector.tensor_copy(o8b[:], psb[:])
            nc.scalar.dma_start(outb_d.ap()[:], o8b[:])

    nc.compile()
    return nc


def kernel(fmap0, fmap1, fmap2, fmap3, fmap4, fc0, fc1, fc2,
           mass0, mass1, mass2, mass3, mass4, mfc, W, b, idx0, idx1):
    from concourse.bass_utils import run_bass_kernel_spmd

    if "nc" not in _CACHE:
        _CACHE["nc"] = _build()
    nc = _CACHE["nc"]

    idx0 = np.asarray(idx0).astype(np.int64)
    idx1 = np.asarray(idx1).astype(np.int64)
    W_ = np.asarray(W, dtype=np.float32).reshape(-1)
    s = np.float32(np.asarray(mfc).reshape(-1)[0])

    # ---- stream A: conv activations + folded V = mass (x) W ----
    # d = tt*2048 + g*256 + i*128 + p
    fmaps = [fmap0, fmap1, fmap2, fmap3, fmap4]
    masses = [mass0, mass1, mass2, mass3, mass4]
    f8 = _f8()
    xa = np.zeros((B, DPA), dtype=f8)
    va = np.zeros(DPA, dtype=np.float32)
    off_w = 0
    off_d = 0
    for (c, h), f, m in zip(CONV, fmaps, masses):
        n = c * h * h
        xa[:, off_d:off_d + n] = np.asarray(f, dtype=np.float32).reshape(B, n)
        m = np.asarray(m, dtype=np.float32)
        va[off_d:off_d + n] = (
            (VSCALE_A * W_[off_w:off_w + c])[:, None, None] * m[None, :, :]
        ).reshape(-1)
        off_w += c
        off_d += n

    xva = np.empty((N_CORES, 128, NDR, TWA), dtype=f8)
    vblk = np.zeros((128, NDR, 2, MD), dtype=f8)
    vblk[:, :, :, :G] = va.reshape(NDR, G, 2, 128).transpose(3, 0, 2, 1)
    xva[:, :, :, :2 * MD] = vblk.reshape(128, NDR, 2 * MD)[None]
    xsrc = xa.reshape(N_CORES, BPC, NDR, G, 2, 128).transpose(0, 5, 2, 4, 3, 1)
    for i in range(2):
        for g in range(G):
            c0 = 2 * MD + i * FREE + g * BPC
            xva[:, :, :, c0:c0 + BPC] = xsrc[:, :, :, i, g, :]
    xva = xva.reshape(N_CORES, 128, NDR * TWA)

    # ---- stream B: gathered fc activations + V = s * W ----
    xb = np.zeros((B, DPB), dtype=np.float16)
    vb = np.zeros(DPB, dtype=np.float32)
    fcs = [(np.asarray(fc0, dtype=np.float32).reshape(B, -1)[:, idx0], FC_MAX),
           (np.asarray(fc1, dtype=np.float32).reshape(B, -1)[:, idx1], FC_MAX),
           (np.asarray(fc2, dtype=np.float32).reshape(B, -1), FC2)]
    off_d = 0
    for data, n in fcs:
        xb[:, off_d:off_d + n] = data
        vb[off_d:off_d + n] = VSCALE_B * s * W_[off_w:off_w + n]
        off_w += n
        off_d += n
    vhb = vb.reshape(NMM_B, G, 128).transpose(2, 0, 1).astype(np.float16)
    xhb = xb.reshape(N_CORES, BPC, NMM_B, G, 128).transpose(0, 4, 2, 3, 1)
    xvb = np.empty((N_CORES, 128, NMM_B, TWB), dtype=np.float16)
    xvb[:, :, :, :G] = vhb[None]
    for g in range(G):
        xvb[:, :, :, G + g * BPC:G + (g + 1) * BPC] = xhb[:, :, :, g, :]
    xvb = xvb.reshape(N_CORES, 128, NMM_B * TWB)

    in_maps = [{"xva": xva[i], "xvb": xvb[i]} for i in range(N_CORES)]

    res = run_bass_kernel_spmd(
        nc, in_maps, core_ids=list(range(N_CORES)), trace=PROFILE
    )
    if PROFILE and res.exec_time_ns is not None:
        print(f"HW exec time: {res.exec_time_ns} ns")
        _CACHE["exec_time_ns"] = res.exec_time_ns
        _CACHE["trace"] = res.instructions_and_trace

    bias = np.float32(np.asarray(b).reshape(-1)[0])
    ia, ib = np.float32(1.0) / VSCALE_A, np.float32(1.0) / VSCALE_B
    rng = np.arange(G)
    out = np.empty((B, 1), dtype=np.float32)
    for i in range(N_CORES):
        da = res.results[i]["oa"].reshape(MD, G, BPC)[rng, rng]
        db = res.results[i]["ob"].reshape(G, G, BPC)[rng, rng]
        out[i * BPC:(i + 1) * BPC, 0] = (
            da.sum(axis=0, dtype=np.float32) * ia
            + db.sum(axis=0, dtype=np.float32) * ib
            + bias
        )
    return out
